# revision 26
# baseline (speedup 1.0000x reference)
"""Trainium2 Bass kernel for NanodetLoss (nn_NanodetLoss_89343909692049).

Strategy
--------
Data-parallel over batch: core r handles images [8r, 8r+8), i.e. a
contiguous 32768-pixel slab of the flattened N = B*H*W axis.

The loss decomposes as
  qfl  = [ sum_{n,c} f(x_nc)  +  sum_{pos} lw*(pos_loss - f(x_at_lab)) ] / num_total
  bbox = 2    * sum_{pos} (1-giou)*wt
  dfl  = 1/16 * sum_{pos,k} dfl_k*wt
  wsum =        sum_{pos} wt
with f(x) = softplus(x)*sigmoid(x)^2 and wt = max_c sigmoid(x) at positives.
Everything except the dense f-sum only matters at the ~2% positive anchors
(labels < 80), so the host compacts the positive rows AND pre-gathers the
per-slot values the positive branch needs (x at the label channel, the 80
channel logits for the wt max) -- all pure indexing; every flop stays on
device. All small per-slot tensors ship as ONE packed [128, 992] f32 DMA
issued before the dense cls chunks.

v3 engine plan (no GPSIMD tensor work, no gathers, no PE transposes):
  ACT:  sigmoid chunks (f32->bf16), then ln(1-s) chunks, with activation
        table loads minimized (Sigmoid, Ln-f32, Ln-bf16 = 3 loads) by
        gating every Ln op on the last sigmoid through a bias-AP token.
        The bbox softmax uses e^b = sig(b)/(1-sig(b)) so no Exp table.
  DVE:  s2 = s*s during the DMA-paced sigmoid phase; in the Ln phase one
        fused scalar_tensor_tensor per chunk computes (s2*1)*ln(1-s) with
        accum_out writing the row sum straight into an accumulator column.
        Geometry (softmax corners, GIoU, DFL prep) and the positive tail
        overlap the dense phase; wt = sigmoid(max of the 80 host-gathered
        channel logits) -- monotonicity makes max-then-sigmoid exact.
Per-core output is a [1,8] partial-sum vector (PE column-sum); the host
adds the 8 vectors and applies the scalar normalizations.
"""

import sys

for _p in ("/opt/trn_rl_repo",):
    if _p not in sys.path:
        sys.path.insert(0, _p)

import numpy as np

import concourse.bass as bass
import concourse.mybir as mybir
from concourse.tile import TileContext
from concourse.vector_clock import ScopedClock
from concourse.bass_utils import run_bass_kernel_spmd

F32 = mybir.dt.float32
BF16 = mybir.dt.bfloat16
AF = mybir.ActivationFunctionType
ALU = mybir.AluOpType
AX = mybir.AxisListType

# Problem geometry (fixed by the task spec).
B, C, R1 = 64, 80, 8
H = W = 64
HW = H * W                 # 4096
NCORES = 8
BPC = B // NCORES          # 8 batches per core
NPC = BPC * HW             # 32768 pixels per core
ROWF = BPC * C * HW // 128  # 20480 elements per SBUF row of the flat cls slab
CH = HW                    # dense chunk size: 4096
NCH = ROWF // CH           # 5
POSCAP = 1024              # padded positive-slot capacity per core
T = POSCAP // 128          # 8 slot columns
REG_TOP = R1 - 1 - 0.1     # 6.9 bbox2distance clamp
EPS = 1e-6

# pack column layout (f32)
PK_BBC = 0            # [0, 256)   bbox logits, T*4*R1
PK_TGT = 256          # [256, 288) bbox targets, T*4
PK_ANC = 288          # [288, 320) anchors, T*4
PK_WV = 320           # [320, 328) valid mask, T
PK_LWV = 328          # [328, 336) label_weights*valid, T
PK_XG = 336           # [336, 344) x at (pixel,label), T
PK_STRD = 344         # [344, 345) stride
PK_CH80 = 352         # [352, 992) the 80 channel logits per slot, T*80
PK_W = 992


class _SplitDrainTileContext(TileContext):
    """This container's walrus build rejects instructions carrying more than
    one sync-wait. Tile's wait assignment freely emits multi-waits, so after
    scheduling we hoist all but one wait of each instruction onto NOPs
    inserted right before it on the same engine (waiting earlier on the same
    engine is equivalent: every hoisted wait was already required there)."""

    def _drain_and_barrier(self, tick_clock, wait_clock):
        drain_inst = self.nc.sync.drain()
        wait_clock.add_sem_waits(
            drain_inst.ins, ScopedClock({None: tick_clock.global_clock})
        )
        waits = list(drain_inst.ins.sync_info.on_wait)
        if len(waits) > 1:
            drain_inst.ins.sync_info.on_wait = waits[:1]
            for w in waits[1:]:
                d2 = self.nc.sync.drain()
                d2.ins.sync_info = mybir.SyncInfo(on_wait=[w], on_update=[])
        self.nc.all_engine_barrier()
        assert self.sems is not None
        popped = self.nc._tile_sem_poison_stack.pop()
        assert popped is self._sem_poison
        self.nc.clear_and_free_semaphores(list(self.sems.allocated().values()))
        self.nc.all_engine_barrier()

    def schedule_and_allocate(self):
        ret = super().schedule_and_allocate()
        nc = self.nc
        for bb_name, bbw in list(nc.bb_map.items()):
            bb = bbw.bb
            insts = bb.instructions
            out = []
            changed = False
            for inst in insts:
                si = inst.sync_info
                if si is not None and si.on_wait and len(si.on_wait) > 1:
                    waits = list(si.on_wait)
                    for w in waits[:-1]:
                        nop = mybir.InstNoOp(
                            name=f"waitnop-{nc.next_id()}",
                            engine=inst.engine,
                            bass_nofuse=True,
                            sync_info=mybir.SyncInfo(on_wait=[w], on_update=[]),
                        )
                        nc.register_instruction(nop)
                        out.append(nop)
                    inst.sync_info = mybir.SyncInfo(
                        on_wait=[waits[-1]], on_update=list(si.on_update))
                    changed = True
                out.append(inst)
            if changed:
                bb.instructions = out
        return ret


def build_nc():
    nc = bass.Bass("TRN2", target_bir_lowering=False, debug=False,
                   num_devices=NCORES)

    cls_d = nc.dram_tensor("cls", [128, ROWF], F32, kind="ExternalInput")
    pack_d = nc.dram_tensor("pack", [128, PK_W], F32, kind="ExternalInput")
    out_d = nc.dram_tensor("out", [128, 8], F32, kind="ExternalOutput")

    with _SplitDrainTileContext(nc) as tc:
        with (
            tc.tile_pool(name="const", bufs=1) as cpool,
            tc.tile_pool(name="xc", bufs=3) as xpool,
            tc.tile_pool(name="sfull", bufs=1) as spool,
            tc.tile_pool(name="s2", bufs=NCH) as s2pool,
            tc.tile_pool(name="sp", bufs=2) as sppool,
            tc.tile_pool(name="scr", bufs=2) as scrpool,
            tc.tile_pool(name="pos", bufs=1) as ppool,
        ):
            # ---------------- DMAs (chunk0, pack, chunks 1-4) ----------
            # chunk0 first so sig0 starts ASAP; pack right behind it so the
            # geometry branch can overlap the rest of the dense stream.
            sfull = spool.tile([128, ROWF], BF16, tag="sfull", name="sfull")
            ssl = [sfull[:, k * CH:(k + 1) * CH] for k in range(NCH)]
            xcs = []

            def chunk_dma(k):
                xk = xpool.tile([128, CH], F32, tag="xchunk", name="xchunk")
                nc.sync.dma_start(out=xk[:], in_=cls_d[:, k * CH:(k + 1) * CH])
                xcs.append(xk)

            # chunk0 ships as two half DMAs so sig0 starts ~3us earlier.
            HH = CH // 2
            x0h = []
            for h in range(2):
                xh = cpool.tile([128, HH], F32, tag=f"x0h{h}", name=f"x0h{h}")
                nc.sync.dma_start(out=xh[:], in_=cls_d[:, h * HH:(h + 1) * HH])
                x0h.append(xh)
            xcs.append(None)
            pack = cpool.tile([128, PK_W], F32, tag="pack", name="pack")
            nc.sync.dma_start(out=pack[:], in_=pack_d[:])
            for k in range(1, NCH):
                chunk_dma(k)

            bbc = pack[:, PK_BBC:PK_BBC + T * 32]
            tgt = pack[:, PK_TGT:PK_TGT + T * 4]
            anc = pack[:, PK_ANC:PK_ANC + T * 4]
            wv = pack[:, PK_WV:PK_WV + T]
            lwv = pack[:, PK_LWV:PK_LWV + T]
            xg = pack[:, PK_XG:PK_XG + T]
            strd = pack[:, PK_STRD:PK_STRD + 1]
            ch80 = pack[:, PK_CH80:PK_CH80 + T * 80]

            # ---------------- constants (gpsimd, tiny) ----------------
            jq8i = cpool.tile([128, T * 32], mybir.dt.int32, tag="jq8i",
                              name="jq8i")
            nc.gpsimd.iota(jq8i[:], pattern=[[0, T], [0, 4], [1, R1]],
                           base=0, channel_multiplier=0)
            jq8 = cpool.tile([128, T * 32], F32, tag="jq8", name="jq8")
            nc.vector.tensor_copy(jq8[:], jq8i[:])

            def vtile(shape, tag):
                return ppool.tile(shape, F32, tag=tag, name=tag)

            def tt(out, a, b, op):
                nc.vector.tensor_tensor(out, a, b, op)

            # ---------------- ACT: sigmoid table block ----------------
            # Dense sig0 halves first (their DMAs land first); the small
            # sigmoids slot into the first DMA gap on the same table.
            # e^b = sig(b)/(1-sig(b)) -- keeps the softmax off the Exp table.
            nc.scalar.activation(ssl[0][:, 0:HH], x0h[0][:], AF.Sigmoid)
            nc.scalar.activation(ssl[0][:, HH:CH], x0h[1][:], AF.Sigmoid)
            sb = vtile([128, T * 32], "sb")
            nc.scalar.activation(sb[:], bbc, AF.Sigmoid)
            sxl = vtile([128, T], "sxl")
            nc.scalar.activation(sxl[:], xg, AF.Sigmoid)

            # wt = sigmoid(max over the 80 channel logits) at positive slots
            wtmx = vtile([128, T], "wtmx")
            nc.vector.tensor_reduce(
                wtmx[:], ch80.rearrange("p (t c) -> p t c", t=T, c=80),
                axis=AX.X, op=ALU.max)
            wt = vtile([128, T], "wt")
            nc.scalar.activation(wt[:], wtmx[:], AF.Sigmoid)
            nc.scalar.activation(ssl[1], xcs[1][:], AF.Sigmoid)
            nc.scalar.activation(ssl[2], xcs[2][:], AF.Sigmoid)

            # ---------------- DVE geometry (overlaps dense DMA) --------
            fin = vtile([128, 8], "fin")
            nc.vector.memset(fin[:], 0.0)
            fac8 = vtile([128, 8], "fac8")
            nc.vector.memset(fac8[:], 0.0)

            omb = vtile([128, T * 32], "omb")
            nc.vector.tensor_scalar(omb[:], sb[:], -1.0, 1.0, ALU.mult,
                                    ALU.add)
            rec = vtile([128, T * 32], "rec")
            nc.vector.reciprocal(rec[:], omb[:])
            e = vtile([128, T * 32], "e")
            tt(e[:], sb[:], rec[:], ALU.mult)

            wtv = vtile([128, T], "wtv")
            tt(wtv[:], wt[:], wv, ALU.mult)
            u2 = vtile([128, T], "u2")
            nc.vector.tensor_scalar(u2[:], sxl[:], -1.0, 1.0, ALU.mult,
                                    ALU.add)

            # centers / normalized targets
            rstr = vtile([128, 1], "rstr")
            nc.vector.reciprocal(rstr[:], strd)
            rsh = vtile([128, 1], "rsh")
            nc.vector.tensor_scalar_mul(rsh[:], rstr[:], 0.5)
            anc3 = anc.rearrange("p (t c) -> p t c", t=T, c=4)
            ctr2 = vtile([128, T * 2], "ctr2")
            ctr2v = ctr2[:].rearrange("p (t c) -> p t c", t=T, c=2)
            tt(ctr2v, anc3[:, :, 0:2], anc3[:, :, 2:4], ALU.add)
            ctr = vtile([128, T * 2], "ctr")
            tt(ctr[:], ctr2[:], rsh[:].broadcast_to((128, T * 2)), ALU.mult)
            targ = vtile([128, T * 4], "targ")
            tt(targ[:], tgt, rstr[:].broadcast_to((128, T * 4)), ALU.mult)

            ctrv = ctr[:].rearrange("p (t c) -> p t c", t=T, c=2)
            targv = targ[:].rearrange("p (t c) -> p t c", t=T, c=4)

            # DFL target distances + tent weights
            dist = vtile([128, T * 4], "dist")
            distv = dist[:].rearrange("p (t c) -> p t c", t=T, c=4)
            tt(distv[:, :, 0:2], ctrv, targv[:, :, 0:2], ALU.subtract)
            tt(distv[:, :, 2:4], targv[:, :, 2:4], ctrv, ALU.subtract)
            nc.vector.tensor_scalar(dist[:], dist[:], 0.0, REG_TOP,
                                    ALU.max, ALU.min)
            y = vtile([128, T * 32], "y")
            tt(y[:].rearrange("p (t k j) -> p t k j", t=T, k=4, j=R1),
               jq8[:].rearrange("p (t k j) -> p t k j", t=T, k=4, j=R1),
               dist[:].rearrange("p (t k) -> p t k", t=T, k=4).unsqueeze(3)
                      .broadcast_to((128, T, 4, R1)),
               ALU.subtract)
            yn = vtile([128, T * 32], "yn")
            nc.vector.tensor_scalar_mul(yn[:], y[:], -1.0)
            ya = vtile([128, T * 32], "ya")
            tt(ya[:], y[:], yn[:], ALU.max)
            tent = vtile([128, T * 32], "tent")
            nc.vector.tensor_scalar(tent[:], ya[:], -1.0, 1.0,
                                    ALU.mult, ALU.add)
            nc.vector.tensor_scalar_max(tent[:], tent[:], 0.0)
            xt = vtile([128, T * 32], "xt")
            tt(xt[:], bbc, tent[:], ALU.mult)
            xts = vtile([128, T * 4], "xts")
            nc.vector.tensor_reduce(
                xts[:].rearrange("p (t k) -> p t k", t=T, k=4),
                xt[:].rearrange("p (t k j) -> p t k j", t=T, k=4, j=R1),
                axis=AX.X, op=ALU.add)

            # softmax integral corners
            S = vtile([128, T * 4], "S")
            nc.vector.tensor_reduce(
                S[:].rearrange("p (t k) -> p t k", t=T, k=4),
                e[:].rearrange("p (t k j) -> p t k j", t=T, k=4, j=R1),
                axis=AX.X, op=ALU.add)
            we = vtile([128, T * 32], "we")
            tt(we[:], e[:], jq8[:], ALU.mult)
            wS = vtile([128, T * 4], "wS")
            nc.vector.tensor_reduce(
                wS[:].rearrange("p (t k) -> p t k", t=T, k=4),
                we[:].rearrange("p (t k j) -> p t k j", t=T, k=4, j=R1),
                axis=AX.X, op=ALU.add)
            rS = vtile([128, T * 4], "rS")
            nc.vector.reciprocal(rS[:], S[:])
            crn = vtile([128, T * 4], "crn")
            tt(crn[:], wS[:], rS[:], ALU.mult)
            crnv = crn[:].rearrange("p (t c) -> p t c", t=T, c=4)

            dec = vtile([128, T * 4], "dec")
            decv = dec[:].rearrange("p (t c) -> p t c", t=T, c=4)
            tt(decv[:, :, 0:2], ctrv, crnv[:, :, 0:2], ALU.subtract)
            tt(decv[:, :, 2:4], ctrv, crnv[:, :, 2:4], ALU.add)

            # aligned IoU + GIoU
            lt = vtile([128, T * 2], "lt")
            tt(lt[:].rearrange("p (t c) -> p t c", t=T, c=2),
               decv[:, :, 0:2], targv[:, :, 0:2], ALU.max)
            rb = vtile([128, T * 2], "rb")
            tt(rb[:].rearrange("p (t c) -> p t c", t=T, c=2),
               decv[:, :, 2:4], targv[:, :, 2:4], ALU.min)
            wh = vtile([128, T * 2], "wh")
            tt(wh[:], rb[:], lt[:], ALU.subtract)
            nc.vector.tensor_scalar_max(wh[:], wh[:], 0.0)
            whv = wh[:].rearrange("p (t c) -> p t c", t=T, c=2)
            ov = vtile([128, T], "ov")
            tt(ov[:].unsqueeze(2), whv[:, :, 0:1], whv[:, :, 1:2], ALU.mult)

            def area(tag, v):
                w_ = vtile([128, T * 2], tag + "wh")
                w_v = w_[:].rearrange("p (t c) -> p t c", t=T, c=2)
                tt(w_v, v[:, :, 2:4], v[:, :, 0:2], ALU.subtract)
                a_ = vtile([128, T], tag)
                tt(a_[:].unsqueeze(2), w_v[:, :, 0:1], w_v[:, :, 1:2], ALU.mult)
                return a_

            ap_ = area("ap", decv)
            at_ = area("at", targv)
            un = vtile([128, T], "un")
            tt(un[:], ap_[:], at_[:], ALU.add)
            tt(un[:], un[:], ov[:], ALU.subtract)
            nc.vector.tensor_scalar_max(un[:], un[:], EPS)
            run_ = vtile([128, T], "run")
            nc.vector.reciprocal(run_[:], un[:])
            iou = vtile([128, T], "iou")
            tt(iou[:], ov[:], run_[:], ALU.mult)

            elt = vtile([128, T * 2], "elt")
            tt(elt[:].rearrange("p (t c) -> p t c", t=T, c=2),
               decv[:, :, 0:2], targv[:, :, 0:2], ALU.min)
            erb = vtile([128, T * 2], "erb")
            tt(erb[:].rearrange("p (t c) -> p t c", t=T, c=2),
               decv[:, :, 2:4], targv[:, :, 2:4], ALU.max)
            ew = vtile([128, T * 2], "ew")
            tt(ew[:], erb[:], elt[:], ALU.subtract)
            nc.vector.tensor_scalar_max(ew[:], ew[:], 0.0)
            ewv = ew[:].rearrange("p (t c) -> p t c", t=T, c=2)
            ea = vtile([128, T], "ea")
            tt(ea[:].unsqueeze(2), ewv[:, :, 0:1], ewv[:, :, 1:2], ALU.mult)
            nc.vector.tensor_scalar_max(ea[:], ea[:], EPS)
            rea = vtile([128, T], "rea")
            nc.vector.reciprocal(rea[:], ea[:])
            gd = vtile([128, T], "gd")
            tt(gd[:], ea[:], un[:], ALU.subtract)
            tt(gd[:], gd[:], rea[:], ALU.mult)
            giou = vtile([128, T], "giou")
            tt(giou[:], iou[:], gd[:], ALU.subtract)
            og = vtile([128, T], "og")
            nc.vector.tensor_scalar(og[:], giou[:], -1.0, 1.0,
                                    ALU.mult, ALU.add)

            # parts of the QFL tail that don't need ln1m
            sxa2 = vtile([128, T], "sxa2")
            tt(sxa2[:], sxl[:], sxl[:], ALU.mult)
            sf = vtile([128, T], "sf")
            tt(sf[:], iou[:], sxl[:], ALU.subtract)
            sf2 = vtile([128, T], "sf2")
            tt(sf2[:], sf[:], sf[:], ALU.mult)
            xsc = vtile([128, T], "xsc")
            tt(xsc[:], xg, iou[:], ALU.mult)

            # ---------------- DVE: first Ln gate + s2 chunks 0-2 ------
            # bias tokens gate Ln ops behind specific sigmoids so the
            # scheduler can't thrash activation tables: the first three Ln
            # chunks slot between sig2 and sig3/4 (one Sigmoid->Ln->Sigmoid
            # round trip), the rest run after the last sigmoid.
            tokA = vtile([128, 1], "tokA")
            nc.vector.tensor_scalar(tokA[:], ssl[2][:, 0:1], 0.0, 1.0,
                                    ALU.mult, ALU.add)
            s2s = []

            def emit_s2(k):
                s2k = s2pool.tile([128, CH], BF16, tag="s2chunk",
                                  name="s2chunk")
                nc.vector.tensor_tensor(s2k[:], ssl[k], ssl[k], ALU.mult)
                s2s.append(s2k)

            for k in range(3):
                emit_s2(k)

            # ---------------- ACT: Ln chunks 0-2, then sig 3-4 --------
            sps = []

            def emit_lnsp(k, tok):
                spk = sppool.tile([128, CH], BF16, tag="spchunk",
                                  name="spchunk")
                nc.scalar.activation(spk[:], ssl[k], AF.Ln,
                                     scale=-1.0, bias=tok[:])
                sps.append(spk)

            emit_lnsp(0, tokA)
            emit_lnsp(1, tokA)
            emit_lnsp(2, tokA)
            nc.scalar.activation(ssl[3], xcs[3][:], AF.Sigmoid)
            nc.scalar.activation(ssl[4], xcs[4][:], AF.Sigmoid)

            # ---------------- DVE: stt 0-2, s2 3-4, second gate -------
            def emit_stt(k):
                scr = scrpool.tile([128, CH], BF16, tag="scr", name="scr")
                nc.vector.scalar_tensor_tensor(
                    scr[:], s2s[k][:], 1.0, sps[k][:], ALU.mult, ALU.mult,
                    accum_out=fac8[:, k:k + 1])

            emit_stt(0)
            emit_stt(1)
            emit_stt(2)
            emit_s2(3)
            tok_z = vtile([128, 1], "tok_z")
            nc.vector.tensor_scalar_mul(tok_z[:], ssl[4][:, 0:1], 0.0)
            tok_o = vtile([128, 1], "tok_o")
            nc.vector.tensor_scalar(tok_o[:], ssl[4][:, 0:1], 0.0, 1.0,
                                    ALU.mult, ALU.add)
            emit_s2(4)

            # ---------------- ACT: remaining Ln block ----------------
            lse = vtile([128, T * 4], "lse")
            nc.scalar.activation(lse[:], S[:], AF.Ln, bias=tok_z[:])
            ln1m = vtile([128, T], "ln1m")
            nc.scalar.activation(ln1m[:], u2[:], AF.Ln, bias=tok_z[:])
            emit_lnsp(3, tok_o)
            emit_lnsp(4, tok_o)

            # ---------------- DVE: ln1m tail (qfl/giou/dfl combine) ----
            gl = vtile([128, 3 * T], "gl")
            spxa = vtile([128, T], "spxa")
            nc.vector.tensor_scalar_mul(spxa[:], ln1m[:], -1.0)
            fxa = vtile([128, T], "fxa")
            tt(fxa[:], sxa2[:], spxa[:], ALU.mult)
            bce = vtile([128, T], "bce")
            tt(bce[:], spxa[:], xsc[:], ALU.subtract)
            pl = vtile([128, T], "pl")
            tt(pl[:], bce[:], sf2[:], ALU.mult)
            qc = vtile([128, T], "qc")
            tt(qc[:], pl[:], fxa[:], ALU.subtract)
            tt(gl[:, 0:T], qc[:], lwv, ALU.mult)

            tt(gl[:, T:2 * T], og[:], wtv[:], ALU.mult)

            dfk = vtile([128, T * 4], "dfk")
            tt(dfk[:], lse[:], xts[:], ALU.subtract)
            dfr = vtile([128, T], "dfr")
            nc.vector.tensor_reduce(
                dfr[:], dfk[:].rearrange("p (t k) -> p t k", t=T, k=4),
                axis=AX.X, op=ALU.add)
            tt(gl[:, 2 * T:3 * T], dfr[:], wtv[:], ALU.mult)

            nc.vector.tensor_reduce(
                fin[:, 1:4], gl[:].rearrange("p (g t) -> p g t", g=3, t=T),
                axis=AX.X, op=ALU.add)
            nc.vector.tensor_reduce(fin[:, 4:5], wtv[:], axis=AX.X,
                                    op=ALU.add)

            # ---------------- DVE: remaining fused fsum chunks --------
            emit_stt(3)
            emit_stt(4)
            # fin col0 keeps the raw (negative) sum of s2*ln(1-s); the host
            # epilogue flips the sign along with the other normalizations.
            nc.vector.tensor_reduce(fin[:, 0:1], fac8[:], axis=AX.X,
                                    op=ALU.add)

            # ---------------- store per-row partials ----------------
            nc.sync.dma_start(out=out_d[:], in_=fin[:])

    return nc


_NC = None


def _get_nc():
    global _NC
    if _NC is None:
        _NC = build_nc()
    return _NC


def make_in_maps(anchors, cls_score, bbox_pred, label_weights, bbox_targets,
                 labels):
    """Host-side sharding + positive-slot compaction/pre-gather.

    Pure indexing only: every arithmetic op of the loss stays on device."""
    cls_score = np.ascontiguousarray(cls_score, np.float32)
    bbox_pred = np.ascontiguousarray(bbox_pred, np.float32)
    labels = np.asarray(labels, np.int32)
    label_weights = np.asarray(label_weights, np.float32)
    bbox_targets = np.asarray(bbox_targets, np.float32)
    anchors = np.asarray(anchors, np.float32)
    cls_flat = cls_score.reshape(B, C, HW)
    bb_flat = bbox_pred.reshape(B, 32, HW)

    def fold(v):  # [POSCAP, k] -> [128, T*k] with slot i = p + 128*t
        k = v.shape[1] if v.ndim > 1 else 1
        return v.reshape(T, 128, k).transpose(1, 0, 2).reshape(128, T * k)

    in_maps = []
    for r in range(NCORES):
        base = r * NPC
        lab = labels[base:base + NPC]
        pos = np.nonzero(lab < C)[0]
        npos = len(pos)
        assert npos <= POSCAP, f"positive count {npos} exceeds cap {POSCAP}"
        idx = np.zeros(POSCAP, np.int64)
        idx[:npos] = pos
        valid = np.zeros(POSCAP, np.float32)
        valid[:npos] = 1.0
        b_loc = idx // HW
        hw = idx % HW
        labp = np.where(valid > 0, lab[idx], 0).astype(np.int64)
        gidx = base + idx
        img = r * BPC + b_loc

        pack = np.zeros((128, PK_W), np.float32)
        pack[:, PK_BBC:PK_BBC + T * 32] = fold(bb_flat[img, :, hw])
        pack[:, PK_TGT:PK_TGT + T * 4] = fold(bbox_targets[gidx])
        pack[:, PK_ANC:PK_ANC + T * 4] = fold(anchors[gidx])
        pack[:, PK_WV:PK_WV + T] = fold(valid[:, None])
        pack[:, PK_LWV:PK_LWV + T] = fold(
            (label_weights[gidx] * valid)[:, None])
        pack[:, PK_XG:PK_XG + T] = fold(cls_flat[img, labp, hw][:, None])
        pack[:, PK_CH80:PK_CH80 + T * 80] = fold(cls_flat[img, :, hw])

        in_maps.append({
            "cls": cls_score[r * BPC:(r + 1) * BPC].reshape(128, ROWF),
            "pack": pack,
        })
    return in_maps


def combine(results, num_total_samples):
    tot = np.zeros(8, np.float64)
    for r in results:
        tot += r["out"].astype(np.float64).sum(axis=0)
    qfl = (-tot[0] + tot[1]) / float(num_total_samples)
    bbox = 2.0 * tot[2]
    dfl = tot[3] * 0.0625
    wsum = tot[4]
    return np.array([qfl, bbox, dfl, wsum], np.float32)


def kernel(anchors, cls_score, bbox_pred, label_weights, bbox_targets,
           labels, num_total_samples, stride):
    in_maps = make_in_maps(anchors, cls_score, bbox_pred, label_weights,
                           bbox_targets, labels)
    for m in in_maps:
        m["pack"][:, PK_STRD] = float(stride)
    nc = _get_nc()
    res = run_bass_kernel_spmd(nc, in_maps, list(range(NCORES)))
    return combine(res.results, num_total_samples)


if __name__ == "__main__":
    pass


# revision 27
# speedup vs baseline: 1.1050x; 1.1050x over previous
"""Trainium2 Bass kernel for NanodetLoss (nn_NanodetLoss_89343909692049).

Strategy
--------
Data-parallel over batch: core r handles images [8r, 8r+8), i.e. a
contiguous 32768-pixel slab of the flattened N = B*H*W axis.

The loss decomposes as
  qfl  = [ sum_{n,c} f(x_nc)  +  sum_{pos} lw*(pos_loss - f(x_at_lab)) ] / num_total
  bbox = 2    * sum_{pos} (1-giou)*wt
  dfl  = 1/16 * sum_{pos,k} dfl_k*wt
  wsum =        sum_{pos} wt
with f(x) = softplus(x)*sigmoid(x)^2 and wt = max_c sigmoid(x) at positives.
Everything except the dense f-sum only matters at the ~2% positive anchors
(labels < 80), so the host compacts the positive rows AND pre-gathers the
per-slot values the positive branch needs (x at the label channel, the 80
channel logits for the wt max) -- all pure indexing; every flop stays on
device. All small per-slot tensors ship as ONE packed [128, 992] f32 DMA
issued before the dense cls chunks.

v3 engine plan (no GPSIMD tensor work, no gathers, no PE transposes):
  ACT:  sigmoid chunks (f32->bf16), then ln(1-s) chunks, with activation
        table loads minimized (Sigmoid, Ln-f32, Ln-bf16 = 3 loads) by
        gating every Ln op on the last sigmoid through a bias-AP token.
        The bbox softmax uses e^b = sig(b)/(1-sig(b)) so no Exp table.
  DVE:  s2 = s*s during the DMA-paced sigmoid phase; in the Ln phase one
        fused scalar_tensor_tensor per chunk computes (s2*1)*ln(1-s) with
        accum_out writing the row sum straight into an accumulator column.
        Geometry (softmax corners, GIoU, DFL prep) and the positive tail
        overlap the dense phase; wt = sigmoid(max of the 80 host-gathered
        channel logits) -- monotonicity makes max-then-sigmoid exact.
Per-core output is a [1,8] partial-sum vector (PE column-sum); the host
adds the 8 vectors and applies the scalar normalizations.
"""

import sys

for _p in ("/opt/trn_rl_repo",):
    if _p not in sys.path:
        sys.path.insert(0, _p)

import numpy as np

import concourse.bass as bass
import concourse.mybir as mybir
from concourse.tile import TileContext
from concourse.vector_clock import ScopedClock
from concourse.bass_utils import run_bass_kernel_spmd

F32 = mybir.dt.float32
BF16 = mybir.dt.bfloat16
AF = mybir.ActivationFunctionType
ALU = mybir.AluOpType
AX = mybir.AxisListType

# Problem geometry (fixed by the task spec).
B, C, R1 = 64, 80, 8
H = W = 64
HW = H * W                 # 4096
NCORES = 8
BPC = B // NCORES          # 8 batches per core
NPC = BPC * HW             # 32768 pixels per core
ROWF = BPC * C * HW // 128  # 20480 elements per SBUF row of the flat cls slab
CH = HW                    # dense chunk size: 4096
NCH = ROWF // CH           # 5
POSCAP = 1024              # padded positive-slot capacity per core
T = POSCAP // 128          # 8 slot columns
REG_TOP = R1 - 1 - 0.1     # 6.9 bbox2distance clamp
EPS = 1e-6

# pack column layout (f32)
PK_BBC = 0            # [0, 256)   bbox logits, T*4*R1
PK_TGT = 256          # [256, 288) bbox targets, T*4
PK_ANC = 288          # [288, 320) anchors, T*4
PK_WV = 320           # [320, 328) valid mask, T
PK_LWV = 328          # [328, 336) label_weights*valid, T
PK_XG = 336           # [336, 344) x at (pixel,label), T
PK_STRD = 344         # [344, 345) stride
PK_CH80 = 352         # [352, 992) the 80 channel logits per slot, T*80
PK_W = 992


class _SplitDrainTileContext(TileContext):
    """This container's walrus build rejects instructions carrying more than
    one sync-wait. Tile's wait assignment freely emits multi-waits, so after
    scheduling we hoist all but one wait of each instruction onto NOPs
    inserted right before it on the same engine (waiting earlier on the same
    engine is equivalent: every hoisted wait was already required there)."""

    def _drain_and_barrier(self, tick_clock, wait_clock):
        drain_inst = self.nc.sync.drain()
        wait_clock.add_sem_waits(
            drain_inst.ins, ScopedClock({None: tick_clock.global_clock})
        )
        waits = list(drain_inst.ins.sync_info.on_wait)
        if len(waits) > 1:
            drain_inst.ins.sync_info.on_wait = waits[:1]
            for w in waits[1:]:
                d2 = self.nc.sync.drain()
                d2.ins.sync_info = mybir.SyncInfo(on_wait=[w], on_update=[])
        self.nc.all_engine_barrier()
        assert self.sems is not None
        popped = self.nc._tile_sem_poison_stack.pop()
        assert popped is self._sem_poison
        self.nc.clear_and_free_semaphores(list(self.sems.allocated().values()))
        self.nc.all_engine_barrier()

    def schedule_and_allocate(self):
        ret = super().schedule_and_allocate()
        nc = self.nc
        for bb_name, bbw in list(nc.bb_map.items()):
            bb = bbw.bb
            insts = bb.instructions
            out = []
            changed = False
            for inst in insts:
                si = inst.sync_info
                if si is not None and si.on_wait and len(si.on_wait) > 1:
                    waits = list(si.on_wait)
                    for w in waits[:-1]:
                        nop = mybir.InstNoOp(
                            name=f"waitnop-{nc.next_id()}",
                            engine=inst.engine,
                            bass_nofuse=True,
                            sync_info=mybir.SyncInfo(on_wait=[w], on_update=[]),
                        )
                        nc.register_instruction(nop)
                        out.append(nop)
                    inst.sync_info = mybir.SyncInfo(
                        on_wait=[waits[-1]], on_update=list(si.on_update))
                    changed = True
                out.append(inst)
            if changed:
                bb.instructions = out
        return ret


def build_nc():
    nc = bass.Bass("TRN2", target_bir_lowering=False, debug=False,
                   num_devices=NCORES)

    cls_d = nc.dram_tensor("cls", [128, ROWF], F32, kind="ExternalInput")
    pack_d = nc.dram_tensor("pack", [128, PK_W], F32, kind="ExternalInput")
    out_d = nc.dram_tensor("out", [128, 8], F32, kind="ExternalOutput")

    with _SplitDrainTileContext(nc) as tc:
        with (
            tc.tile_pool(name="const", bufs=1) as cpool,
            tc.tile_pool(name="xc", bufs=3) as xpool,
            tc.tile_pool(name="sfull", bufs=1) as spool,
            tc.tile_pool(name="s2", bufs=NCH) as s2pool,
            tc.tile_pool(name="sp", bufs=3) as sppool,
            tc.tile_pool(name="scr", bufs=2) as scrpool,
            tc.tile_pool(name="pos", bufs=1) as ppool,
        ):
            # ---------------- DMAs (chunk0, pack, chunks 1-4) ----------
            # chunk0 first so sig0 starts ASAP; pack right behind it so the
            # geometry branch can overlap the rest of the dense stream.
            sfull = spool.tile([128, ROWF], BF16, tag="sfull", name="sfull")
            ssl = [sfull[:, k * CH:(k + 1) * CH] for k in range(NCH)]
            xcs = []

            def chunk_dma(k):
                xk = xpool.tile([128, CH], F32, tag="xchunk", name="xchunk")
                nc.sync.dma_start(out=xk[:], in_=cls_d[:, k * CH:(k + 1) * CH])
                xcs.append(xk)

            # chunk0 ships as two half DMAs so sig0 starts ~3us earlier.
            HH = CH // 2
            x0h = []
            for h in range(2):
                xh = cpool.tile([128, HH], F32, tag=f"x0h{h}", name=f"x0h{h}")
                nc.sync.dma_start(out=xh[:], in_=cls_d[:, h * HH:(h + 1) * HH])
                x0h.append(xh)
            xcs.append(None)
            pack = cpool.tile([128, PK_W], F32, tag="pack", name="pack")
            nc.sync.dma_start(out=pack[:], in_=pack_d[:])
            for k in range(1, NCH):
                chunk_dma(k)

            bbc = pack[:, PK_BBC:PK_BBC + T * 32]
            tgt = pack[:, PK_TGT:PK_TGT + T * 4]
            anc = pack[:, PK_ANC:PK_ANC + T * 4]
            wv = pack[:, PK_WV:PK_WV + T]
            lwv = pack[:, PK_LWV:PK_LWV + T]
            xg = pack[:, PK_XG:PK_XG + T]
            strd = pack[:, PK_STRD:PK_STRD + 1]
            ch80 = pack[:, PK_CH80:PK_CH80 + T * 80]

            # ---------------- constants (gpsimd, tiny) ----------------
            jq8i = cpool.tile([128, T * 32], mybir.dt.int32, tag="jq8i",
                              name="jq8i")
            nc.gpsimd.iota(jq8i[:], pattern=[[0, T], [0, 4], [1, R1]],
                           base=0, channel_multiplier=0)
            jq8 = cpool.tile([128, T * 32], F32, tag="jq8", name="jq8")
            nc.vector.tensor_copy(jq8[:], jq8i[:])

            def vtile(shape, tag):
                return ppool.tile(shape, F32, tag=tag, name=tag)

            def tt(out, a, b, op):
                nc.vector.tensor_tensor(out, a, b, op)

            # ---------------- ACT: sigmoid table block ----------------
            # Dense sig0 halves first (their DMAs land first); the small
            # sigmoids slot into the first DMA gap on the same table.
            # e^b = sig(b)/(1-sig(b)) -- keeps the softmax off the Exp table.
            nc.scalar.activation(ssl[0][:, 0:HH], x0h[0][:], AF.Sigmoid)
            nc.scalar.activation(ssl[0][:, HH:CH], x0h[1][:], AF.Sigmoid)
            sb = vtile([128, T * 32], "sb")
            nc.scalar.activation(sb[:], bbc, AF.Sigmoid)
            sxl = vtile([128, T], "sxl")
            nc.scalar.activation(sxl[:], xg, AF.Sigmoid)

            # wt = sigmoid(max over the 80 channel logits) at positive slots
            wtmx = vtile([128, T], "wtmx")
            nc.vector.tensor_reduce(
                wtmx[:], ch80.rearrange("p (t c) -> p t c", t=T, c=80),
                axis=AX.X, op=ALU.max)
            wt = vtile([128, T], "wt")
            nc.scalar.activation(wt[:], wtmx[:], AF.Sigmoid)
            nc.scalar.activation(ssl[1], xcs[1][:], AF.Sigmoid)
            nc.scalar.activation(ssl[2], xcs[2][:], AF.Sigmoid)

            # ---------------- DVE geometry (overlaps dense DMA) --------
            fin = vtile([128, 8], "fin")
            nc.vector.memset(fin[:], 0.0)
            fac8 = vtile([128, 8], "fac8")
            nc.vector.memset(fac8[:], 0.0)

            omb = vtile([128, T * 32], "omb")
            nc.vector.tensor_scalar(omb[:], sb[:], -1.0, 1.0, ALU.mult,
                                    ALU.add)
            rec = vtile([128, T * 32], "rec")
            nc.vector.reciprocal(rec[:], omb[:])
            e = vtile([128, T * 32], "e")
            tt(e[:], sb[:], rec[:], ALU.mult)

            wtv = vtile([128, T], "wtv")
            tt(wtv[:], wt[:], wv, ALU.mult)
            u2 = vtile([128, T], "u2")
            nc.vector.tensor_scalar(u2[:], sxl[:], -1.0, 1.0, ALU.mult,
                                    ALU.add)

            # centers / normalized targets
            rstr = vtile([128, 1], "rstr")
            nc.vector.reciprocal(rstr[:], strd)
            rsh = vtile([128, 1], "rsh")
            nc.vector.tensor_scalar_mul(rsh[:], rstr[:], 0.5)
            anc3 = anc.rearrange("p (t c) -> p t c", t=T, c=4)
            ctr2 = vtile([128, T * 2], "ctr2")
            ctr2v = ctr2[:].rearrange("p (t c) -> p t c", t=T, c=2)
            tt(ctr2v, anc3[:, :, 0:2], anc3[:, :, 2:4], ALU.add)
            ctr = vtile([128, T * 2], "ctr")
            tt(ctr[:], ctr2[:], rsh[:].broadcast_to((128, T * 2)), ALU.mult)
            targ = vtile([128, T * 4], "targ")
            tt(targ[:], tgt, rstr[:].broadcast_to((128, T * 4)), ALU.mult)

            ctrv = ctr[:].rearrange("p (t c) -> p t c", t=T, c=2)
            targv = targ[:].rearrange("p (t c) -> p t c", t=T, c=4)

            # DFL target distances + tent weights
            dist = vtile([128, T * 4], "dist")
            distv = dist[:].rearrange("p (t c) -> p t c", t=T, c=4)
            tt(distv[:, :, 0:2], ctrv, targv[:, :, 0:2], ALU.subtract)
            tt(distv[:, :, 2:4], targv[:, :, 2:4], ctrv, ALU.subtract)
            nc.vector.tensor_scalar(dist[:], dist[:], 0.0, REG_TOP,
                                    ALU.max, ALU.min)
            y = vtile([128, T * 32], "y")
            tt(y[:].rearrange("p (t k j) -> p t k j", t=T, k=4, j=R1),
               jq8[:].rearrange("p (t k j) -> p t k j", t=T, k=4, j=R1),
               dist[:].rearrange("p (t k) -> p t k", t=T, k=4).unsqueeze(3)
                      .broadcast_to((128, T, 4, R1)),
               ALU.subtract)
            yn = vtile([128, T * 32], "yn")
            nc.vector.tensor_scalar_mul(yn[:], y[:], -1.0)
            ya = vtile([128, T * 32], "ya")
            tt(ya[:], y[:], yn[:], ALU.max)
            tent = vtile([128, T * 32], "tent")
            nc.vector.tensor_scalar(tent[:], ya[:], -1.0, 1.0,
                                    ALU.mult, ALU.add)
            nc.vector.tensor_scalar_max(tent[:], tent[:], 0.0)
            xt = vtile([128, T * 32], "xt")
            tt(xt[:], bbc, tent[:], ALU.mult)
            xts = vtile([128, T * 4], "xts")
            nc.vector.tensor_reduce(
                xts[:].rearrange("p (t k) -> p t k", t=T, k=4),
                xt[:].rearrange("p (t k j) -> p t k j", t=T, k=4, j=R1),
                axis=AX.X, op=ALU.add)

            # softmax integral corners
            S = vtile([128, T * 4], "S")
            nc.vector.tensor_reduce(
                S[:].rearrange("p (t k) -> p t k", t=T, k=4),
                e[:].rearrange("p (t k j) -> p t k j", t=T, k=4, j=R1),
                axis=AX.X, op=ALU.add)
            we = vtile([128, T * 32], "we")
            tt(we[:], e[:], jq8[:], ALU.mult)
            wS = vtile([128, T * 4], "wS")
            nc.vector.tensor_reduce(
                wS[:].rearrange("p (t k) -> p t k", t=T, k=4),
                we[:].rearrange("p (t k j) -> p t k j", t=T, k=4, j=R1),
                axis=AX.X, op=ALU.add)
            rS = vtile([128, T * 4], "rS")
            nc.vector.reciprocal(rS[:], S[:])
            crn = vtile([128, T * 4], "crn")
            tt(crn[:], wS[:], rS[:], ALU.mult)
            crnv = crn[:].rearrange("p (t c) -> p t c", t=T, c=4)

            dec = vtile([128, T * 4], "dec")
            decv = dec[:].rearrange("p (t c) -> p t c", t=T, c=4)
            tt(decv[:, :, 0:2], ctrv, crnv[:, :, 0:2], ALU.subtract)
            tt(decv[:, :, 2:4], ctrv, crnv[:, :, 2:4], ALU.add)

            # aligned IoU + GIoU
            lt = vtile([128, T * 2], "lt")
            tt(lt[:].rearrange("p (t c) -> p t c", t=T, c=2),
               decv[:, :, 0:2], targv[:, :, 0:2], ALU.max)
            rb = vtile([128, T * 2], "rb")
            tt(rb[:].rearrange("p (t c) -> p t c", t=T, c=2),
               decv[:, :, 2:4], targv[:, :, 2:4], ALU.min)
            wh = vtile([128, T * 2], "wh")
            tt(wh[:], rb[:], lt[:], ALU.subtract)
            nc.vector.tensor_scalar_max(wh[:], wh[:], 0.0)
            whv = wh[:].rearrange("p (t c) -> p t c", t=T, c=2)
            ov = vtile([128, T], "ov")
            tt(ov[:].unsqueeze(2), whv[:, :, 0:1], whv[:, :, 1:2], ALU.mult)

            def area(tag, v):
                w_ = vtile([128, T * 2], tag + "wh")
                w_v = w_[:].rearrange("p (t c) -> p t c", t=T, c=2)
                tt(w_v, v[:, :, 2:4], v[:, :, 0:2], ALU.subtract)
                a_ = vtile([128, T], tag)
                tt(a_[:].unsqueeze(2), w_v[:, :, 0:1], w_v[:, :, 1:2], ALU.mult)
                return a_

            ap_ = area("ap", decv)
            at_ = area("at", targv)
            un = vtile([128, T], "un")
            tt(un[:], ap_[:], at_[:], ALU.add)
            tt(un[:], un[:], ov[:], ALU.subtract)
            nc.vector.tensor_scalar_max(un[:], un[:], EPS)
            run_ = vtile([128, T], "run")
            nc.vector.reciprocal(run_[:], un[:])
            iou = vtile([128, T], "iou")
            tt(iou[:], ov[:], run_[:], ALU.mult)

            elt = vtile([128, T * 2], "elt")
            tt(elt[:].rearrange("p (t c) -> p t c", t=T, c=2),
               decv[:, :, 0:2], targv[:, :, 0:2], ALU.min)
            erb = vtile([128, T * 2], "erb")
            tt(erb[:].rearrange("p (t c) -> p t c", t=T, c=2),
               decv[:, :, 2:4], targv[:, :, 2:4], ALU.max)
            ew = vtile([128, T * 2], "ew")
            tt(ew[:], erb[:], elt[:], ALU.subtract)
            nc.vector.tensor_scalar_max(ew[:], ew[:], 0.0)
            ewv = ew[:].rearrange("p (t c) -> p t c", t=T, c=2)
            ea = vtile([128, T], "ea")
            tt(ea[:].unsqueeze(2), ewv[:, :, 0:1], ewv[:, :, 1:2], ALU.mult)
            nc.vector.tensor_scalar_max(ea[:], ea[:], EPS)
            rea = vtile([128, T], "rea")
            nc.vector.reciprocal(rea[:], ea[:])
            gd = vtile([128, T], "gd")
            tt(gd[:], ea[:], un[:], ALU.subtract)
            tt(gd[:], gd[:], rea[:], ALU.mult)
            giou = vtile([128, T], "giou")
            tt(giou[:], iou[:], gd[:], ALU.subtract)
            og = vtile([128, T], "og")
            nc.vector.tensor_scalar(og[:], giou[:], -1.0, 1.0,
                                    ALU.mult, ALU.add)

            # parts of the QFL tail that don't need ln1m
            sxa2 = vtile([128, T], "sxa2")
            tt(sxa2[:], sxl[:], sxl[:], ALU.mult)
            sf = vtile([128, T], "sf")
            tt(sf[:], iou[:], sxl[:], ALU.subtract)
            sf2 = vtile([128, T], "sf2")
            tt(sf2[:], sf[:], sf[:], ALU.mult)
            xsc = vtile([128, T], "xsc")
            tt(xsc[:], xg, iou[:], ALU.mult)

            # ---------------- DVE: first Ln gate + s2 chunks 0-2 ------
            # bias tokens gate Ln ops behind specific sigmoids so the
            # scheduler can't thrash activation tables: the first three Ln
            # chunks slot between sig2 and sig3/4 (one Sigmoid->Ln->Sigmoid
            # round trip), the rest run after the last sigmoid.
            tokA = vtile([128, 1], "tokA")
            nc.vector.tensor_scalar(tokA[:], ssl[2][:, 0:1], 0.0, 1.0,
                                    ALU.mult, ALU.add)
            s2s = []

            def emit_s2(k):
                s2k = s2pool.tile([128, CH], BF16, tag="s2chunk",
                                  name="s2chunk")
                nc.vector.tensor_tensor(s2k[:], ssl[k], ssl[k], ALU.mult)
                s2s.append(s2k)

            for k in range(3):
                emit_s2(k)

            # ---------------- ACT: Ln chunks 0-2, then sig 3-4 --------
            sps = []

            def emit_lnsp(k, tok):
                spk = sppool.tile([128, CH], BF16, tag="spchunk",
                                  name="spchunk")
                nc.scalar.activation(spk[:], ssl[k], AF.Ln,
                                     scale=-1.0, bias=tok[:])
                sps.append(spk)

            emit_lnsp(0, tokA)
            emit_lnsp(1, tokA)
            emit_lnsp(2, tokA)
            nc.scalar.activation(ssl[3], xcs[3][:], AF.Sigmoid)
            nc.scalar.activation(ssl[4], xcs[4][:], AF.Sigmoid)

            # ---------------- DVE: stt 0-2, s2 3-4, second gate -------
            def emit_stt(k):
                scr = scrpool.tile([128, CH], BF16, tag="scr", name="scr")
                nc.vector.scalar_tensor_tensor(
                    scr[:], s2s[k][:], 1.0, sps[k][:], ALU.mult, ALU.mult,
                    accum_out=fac8[:, k:k + 1])

            emit_stt(0)
            emit_stt(1)
            emit_stt(2)
            emit_s2(3)
            tok_z = vtile([128, 1], "tok_z")
            nc.vector.tensor_scalar_mul(tok_z[:], ssl[4][:, 0:1], 0.0)
            tok_o = vtile([128, 1], "tok_o")
            nc.vector.tensor_scalar(tok_o[:], ssl[4][:, 0:1], 0.0, 1.0,
                                    ALU.mult, ALU.add)
            emit_s2(4)

            # ---------------- ACT: remaining Ln block ----------------
            lse = vtile([128, T * 4], "lse")
            nc.scalar.activation(lse[:], S[:], AF.Ln, bias=tok_z[:])
            ln1m = vtile([128, T], "ln1m")
            nc.scalar.activation(ln1m[:], u2[:], AF.Ln, bias=tok_z[:])
            emit_lnsp(3, tok_o)
            emit_lnsp(4, tok_o)

            # ---------------- DVE: ln1m tail (qfl/giou/dfl combine) ----
            gl = vtile([128, 3 * T], "gl")
            spxa = vtile([128, T], "spxa")
            nc.vector.tensor_scalar_mul(spxa[:], ln1m[:], -1.0)
            fxa = vtile([128, T], "fxa")
            tt(fxa[:], sxa2[:], spxa[:], ALU.mult)
            bce = vtile([128, T], "bce")
            tt(bce[:], spxa[:], xsc[:], ALU.subtract)
            pl = vtile([128, T], "pl")
            tt(pl[:], bce[:], sf2[:], ALU.mult)
            qc = vtile([128, T], "qc")
            tt(qc[:], pl[:], fxa[:], ALU.subtract)
            tt(gl[:, 0:T], qc[:], lwv, ALU.mult)

            tt(gl[:, T:2 * T], og[:], wtv[:], ALU.mult)

            dfk = vtile([128, T * 4], "dfk")
            tt(dfk[:], lse[:], xts[:], ALU.subtract)
            dfr = vtile([128, T], "dfr")
            nc.vector.tensor_reduce(
                dfr[:], dfk[:].rearrange("p (t k) -> p t k", t=T, k=4),
                axis=AX.X, op=ALU.add)
            tt(gl[:, 2 * T:3 * T], dfr[:], wtv[:], ALU.mult)

            nc.vector.tensor_reduce(
                fin[:, 1:4], gl[:].rearrange("p (g t) -> p g t", g=3, t=T),
                axis=AX.X, op=ALU.add)
            nc.vector.tensor_reduce(fin[:, 4:5], wtv[:], axis=AX.X,
                                    op=ALU.add)

            # ---------------- DVE: remaining fused fsum chunks --------
            emit_stt(3)
            emit_stt(4)
            # fin col0 keeps the raw (negative) sum of s2*ln(1-s); the host
            # epilogue flips the sign along with the other normalizations.
            nc.vector.tensor_reduce(fin[:, 0:1], fac8[:], axis=AX.X,
                                    op=ALU.add)

            # ---------------- store per-row partials ----------------
            nc.sync.dma_start(out=out_d[:], in_=fin[:])

    return nc


_NC = None


def _get_nc():
    global _NC
    if _NC is None:
        _NC = build_nc()
    return _NC


def make_in_maps(anchors, cls_score, bbox_pred, label_weights, bbox_targets,
                 labels):
    """Host-side sharding + positive-slot compaction/pre-gather.

    Pure indexing only: every arithmetic op of the loss stays on device."""
    cls_score = np.ascontiguousarray(cls_score, np.float32)
    bbox_pred = np.ascontiguousarray(bbox_pred, np.float32)
    labels = np.asarray(labels, np.int32)
    label_weights = np.asarray(label_weights, np.float32)
    bbox_targets = np.asarray(bbox_targets, np.float32)
    anchors = np.asarray(anchors, np.float32)
    cls_flat = cls_score.reshape(B, C, HW)
    bb_flat = bbox_pred.reshape(B, 32, HW)

    def fold(v):  # [POSCAP, k] -> [128, T*k] with slot i = p + 128*t
        k = v.shape[1] if v.ndim > 1 else 1
        return v.reshape(T, 128, k).transpose(1, 0, 2).reshape(128, T * k)

    in_maps = []
    for r in range(NCORES):
        base = r * NPC
        lab = labels[base:base + NPC]
        pos = np.nonzero(lab < C)[0]
        npos = len(pos)
        assert npos <= POSCAP, f"positive count {npos} exceeds cap {POSCAP}"
        idx = np.zeros(POSCAP, np.int64)
        idx[:npos] = pos
        valid = np.zeros(POSCAP, np.float32)
        valid[:npos] = 1.0
        b_loc = idx // HW
        hw = idx % HW
        labp = np.where(valid > 0, lab[idx], 0).astype(np.int64)
        gidx = base + idx
        img = r * BPC + b_loc

        pack = np.zeros((128, PK_W), np.float32)
        pack[:, PK_BBC:PK_BBC + T * 32] = fold(bb_flat[img, :, hw])
        pack[:, PK_TGT:PK_TGT + T * 4] = fold(bbox_targets[gidx])
        pack[:, PK_ANC:PK_ANC + T * 4] = fold(anchors[gidx])
        pack[:, PK_WV:PK_WV + T] = fold(valid[:, None])
        pack[:, PK_LWV:PK_LWV + T] = fold(
            (label_weights[gidx] * valid)[:, None])
        pack[:, PK_XG:PK_XG + T] = fold(cls_flat[img, labp, hw][:, None])
        pack[:, PK_CH80:PK_CH80 + T * 80] = fold(cls_flat[img, :, hw])

        in_maps.append({
            "cls": cls_score[r * BPC:(r + 1) * BPC].reshape(128, ROWF),
            "pack": pack,
        })
    return in_maps


def combine(results, num_total_samples):
    tot = np.zeros(8, np.float64)
    for r in results:
        tot += r["out"].astype(np.float64).sum(axis=0)
    qfl = (-tot[0] + tot[1]) / float(num_total_samples)
    bbox = 2.0 * tot[2]
    dfl = tot[3] * 0.0625
    wsum = tot[4]
    return np.array([qfl, bbox, dfl, wsum], np.float32)


def kernel(anchors, cls_score, bbox_pred, label_weights, bbox_targets,
           labels, num_total_samples, stride):
    in_maps = make_in_maps(anchors, cls_score, bbox_pred, label_weights,
                           bbox_targets, labels)
    for m in in_maps:
        m["pack"][:, PK_STRD] = float(stride)
    nc = _get_nc()
    res = run_bass_kernel_spmd(nc, in_maps, list(range(NCORES)))
    return combine(res.results, num_total_samples)


if __name__ == "__main__":
    pass
